# revision 139
# baseline (speedup 1.0000x reference)
"""Trainium2 Bass kernel for nn_LocalAttention (5x5 local window attention).

Contract: kernel(**inputs) takes the FULL inputs from setup_inputs() and
returns the FULL output.  Internally shards across 8 NeuronCores as
(batch b in 0..3) x (head-group hg in 0..1, 4 heads each).  Each core
computes a partial output projection; the host sums the two partials per
batch and adds b_out once.

Per-core algorithm (validated against the reference in numpy):
  - qT,kT (d-major, fp16) and v (pixel-major, fp16, with ones column for
    the softmax denominator) via fp16 matmuls from host-pre-transposed
    x.T and w slices.
  - k/v live in buffers padded with 2 zero image-rows top+bottom
    (buffer pixel = image pixel + 128): padded neighbors naturally give
    dots=0 -> exp(0)=1 in the denominator and v=0, matching the
    reference's zero-padded local window.
  - Per 256-pixel batch s: banded transposed pairwise dots
    E_T[j, p] = k_buf[s+j] . q[s+p], 2-head row-group-packed matmuls
    (fp16 in, fp32 psum).  Band chunk 0 only feeds pixel-half 0 and
    chunk 3 only half 1 (everything else is window/column-masked), so
    the two edge chunks share one half-width psum tile and the AV
    accumulation skips the provably-masked (chunk, half) pairs.
  - exp on ACT (scale=1/8); DVE multiplies by a precomputed 0/1
    window/wrap mask (fp16 2x mode).  Column-wrapped neighbors are
    masked out and re-added to the denominator via n_pad.
  - Weighted sum over v + denominator via the ones column, one
    accumulating matmul chain per (head, pixel-half).
  - Normalize (DVE reciprocal + per-partition scalar multiply into fp16
    opix), transpose O on the PE (fp16 identity), partial
    out-projection; output DMA'd as fp16 partials.

Schedule: phase B (projections) is software-pipelined into phase C in
half-blocks; each attention batch si is split into a front half
(dots/exp/mask/AV/den/rec/rescale) and a back half (transpose/copies/
projection/DMA) emitted SKEW batches later so no engine head-of-line
blocks on the output chain.  PSUM lives in one 8-bank pool: pw 2x2
banks (shared by B's q/k matmuls' reuse, the dots, and the projection),
a 3-deep po rotation (AV accumulators + B's v matmuls), and one pt bank
for the transposes.  Engine assignment and orderings were swept against
TimelineSim (see sweep.py); knobs remain env-overridable.
"""

import numpy as np

B, HMAP, WMAP = 4, 64, 64
N = HMAP * WMAP          # 4096
DIM = 512
HEADS, HEAD_DIM = 8, 64
INNER = HEADS * HEAD_DIM  # 512
SCALE = HEAD_DIM ** -0.5
NB = N + 256             # padded k/v buffer pixels (2 zero rows each side)
NCHUNK = NB // 128       # 34
N_CORES = 8

_cache = {}


def _make_masks():
    """Window/wrap masks, edge-merged: row 0 = chunk0 cols [0:128) | chunk3
    cols [128:256), rows 1/2 = full middle chunks 1/2.  Plus n_pad^T.

    mask[c, j', p'] = 1 iff o = 128*c + j' - p' - 128 decomposes as
    64*di + dj with |di|,|dj| <= 2 and column p'%64 + dj stays in-image.
    n_pad[p] = number of column-invalid window positions for column p%64.
    """
    o = (128 * np.arange(4)[:, None, None] + np.arange(128)[None, :, None]
         - np.arange(256)[None, None, :] - 128)           # [4,128,256]
    di = np.round(o / 64.0).astype(np.int64)
    dj = o - 64 * di
    col = (np.arange(256) % 64)[None, None, :]
    ok = (np.abs(di) <= 2) & (np.abs(dj) <= 2) & (col + dj >= 0) & (col + dj < 64)
    # log-domain: 0 where valid, -160 where masked (exp -> ~0 after /8 scale)
    m4 = np.where(ok, 0.0, -160.0).astype(np.float16)
    edge = np.concatenate([m4[0][:, 0:128], m4[3][:, 128:256]], axis=1)
    masks = np.stack([edge, m4[1], m4[2]])                 # [3,128,256]
    m01 = ok.astype(np.float16)
    edge01 = np.concatenate([m01[0][:, 0:128], m01[3][:, 128:256]], axis=1)
    masks01 = np.stack([edge01, m01[1], m01[2]])
    colv = np.arange(64)
    npad_col = np.zeros(64, dtype=np.float32)
    for djv in range(-2, 3):
        npad_col += 5.0 * ((colv + djv < 0) | (colv + djv >= 64))
    npadt = np.tile(npad_col, 2).reshape(1, 128).astype(np.float16)
    return masks, masks01, npadt


def _build_nc(stage=99):
    import os
    stage = int(os.environ.get("KSTAGE", stage))
    # schedule knobs (sim-swept): see sweep.py
    EDGE_FIRST = os.environ.get("K_EDGE", "first") == "first"
    HALF_AFTER = os.environ.get("K_HALF", "after") == "after"
    XTILE_GP = os.environ.get("K_XQ", "sync") == "gp"
    RESCALE_FRONT = os.environ.get("K_RESC", "back") == "front"
    MASKE = os.environ.get("K_MASKE", "dve")       # pool | dve | split
    OB = os.environ.get("K_OB", "act")             # mix | dve | act
    OTB = os.environ.get("K_OTB", "dve")           # dve | act | mix
    RESC_ENG = os.environ.get("K_RESCE", "dve")    # dve | act | mix
    SKEW = int(os.environ.get("K_SKEW", "4"))
    EMB = int(os.environ.get("K_EMB", "3"))
    ERB = int(os.environ.get("K_ERB", "14"))
    OPB = int(os.environ.get("K_OPB", "3"))
    FOLD_E = os.environ.get("K_FOLDE", "dve") == "pe"  # edge mask: pe | dve
    FOLD_M = os.environ.get("K_FOLDM", "dve") == "pe"  # mid mask: pe | dve
    VCP = os.environ.get("K_VCP", "dve")           # mix | act | dve
    QKCP = os.environ.get("K_QKCP", "qd_ka")       # qa_kd | qd_ka
    XINB = int(os.environ.get("K_XINB", "3"))
    OTBB = int(os.environ.get("K_OTBB", "3"))
    OBB = int(os.environ.get("K_OBB", "5"))
    RESC_MERGE = os.environ.get("K_RESCM", "0") == "1"
    if SKEW >= 2:
        # po psum tags are only 2-deep; deeper skew must free them in front
        RESCALE_FRONT = True
        OPB = max(OPB, SKEW + 2)
    import concourse.bass as bass
    import concourse.tile as tile
    from concourse import mybir

    f32 = mybir.dt.float32
    f32r = mybir.dt.float32r
    f16 = mybir.dt.float16
    Exp = mybir.ActivationFunctionType.Exp

    from concourse import bacc
    nc = bacc.Bacc(None, target_bir_lowering=False)
    xt_d = nc.dram_tensor("xt", [DIM, N], f16, kind="ExternalInput")
    wqkvt_d = nc.dram_tensor("wqkvt", [DIM, 768], f16, kind="ExternalInput")
    woutt_d = nc.dram_tensor("woutt", [256, DIM], f16, kind="ExternalInput")
    masks_d = nc.dram_tensor("masks", [3, 128, 256], f16, kind="ExternalInput")
    npad_d = nc.dram_tensor("npad", [128, 1], f32, kind="ExternalInput")
    ident_d = nc.dram_tensor("ident", [128, 128], f16, kind="ExternalInput")
    out_d = nc.dram_tensor("out", [N, DIM], f16, kind="ExternalOutput")

    def r32(ap):
        return ap.bitcast(f32r)

    with tile.TileContext(nc) as tc:
        from contextlib import ExitStack
        with ExitStack() as ctx:
            consts = ctx.enter_context(tc.tile_pool(name="consts", bufs=1))

            # Startup is paced by the single HWDGE's 625ns-per-DMA dispatch:
            # only block 0's operand slices go ahead of the first x tile;
            # everything else is deferred behind it.
            wqkvt = consts.tile([128, 4, 768], f16)
            wq_view = wqkvt_d.rearrange("(c p) m -> p c m", p=128)
            nc.sync.dma_start(out=wqkvt[:, :, 0:256],
                              in_=wq_view[:, :, 0:256])
            nc.sync.dma_start(out=wqkvt[:, :, 512:768],
                              in_=wq_view[:, :, 512:768])
            nc.sync.dma_start(out=wqkvt[:, :, 256:512],
                              in_=wq_view[:, :, 256:512])
            # masks content is host-prepared per fold mode: log-bias rows for
            # PE-folded chunks, 0/1 rows for DVE-multiplied chunks
            woutt = consts.tile([128, 2, DIM], f16)
            masks = consts.tile([128, 3, 256], f16)
            masks01 = masks
            npad = consts.tile([128, 1], f32)
            ident = consts.tile([128, 128], f16)
            nc.sync.dma_start(out=ident, in_=ident_d[:, :])

            def emit_wq_k():
                pass

            def emit_late_consts():
                # deferred consts aren't needed until attention starts; keep
                # them off the sync queue so the first x tiles aren't blocked
                cq = {"sync": nc.sync, "dve": nc.vector,
                      "act": nc.scalar}[os.environ.get("K_CQ", "sync")]
                cq.dma_start(out=woutt,
                             in_=woutt_d.rearrange("(c p) m -> p c m",
                                                   p=128))
                cq.dma_start(out=masks,
                             in_=masks_d.rearrange("c p f -> p c f"))
                cq.dma_start(out=npad, in_=npad_d[:, :])

            # persistent activations
            qt = [consts.tile([128, N], f16, tag=f"qt{g}", name=f"qt{g}") for g in range(2)]
            kt = [consts.tile([128, NB], f16, tag=f"kt{g}", name=f"kt{g}") for g in range(2)]
            # v buffer: [p, chunk, 4 heads x (64 + ones col)]
            vsb = consts.tile([128, NCHUNK, 260], f16)

            for g in range(2):
                nc.vector.memset(kt[g][:, 0:128], 0.0)
                nc.vector.memset(kt[g][:, NB - 128:NB], 0.0)
            nc.vector.memset(vsb[:, 0, :], 0.0)
            nc.vector.memset(vsb[:, NCHUNK - 1, :], 0.0)
            # ones columns (after zero memsets of the pad chunks)
            ones_ap = vsb.rearrange("p c (h e) -> p c h e", h=4)[:, :, :, 64:65]
            nc.vector.memset(ones_ap, 1.0)

            # One PSUM pool shared by phases B and C (8 banks total):
            #   pw      : 2 bufs x [128,2,2,256] f32 (2 banks each)  -> 4
            #   po0/1/2 : 1 buf  x [128,4,65]    f32                 -> 3
            #   pt      : 1 buf  x [128,4,128]   f16                 -> 1
            # (pj reuses the pw slots; psqk/psv reuse pw/po slots)
            psum = ctx.enter_context(
                tc.tile_pool(name="psum", bufs=1, space="PSUM"))
            po_ctr = [0]

            def po_tag():
                po_ctr[0] += 1
                return f"po{po_ctr[0] % 3}"

            # ---------------- Phases B & C, software-pipelined ----------------
            xin = ctx.enter_context(tc.tile_pool(name="xin", bufs=XINB))
            epool = ctx.enter_context(tc.tile_pool(name="em", bufs=EMB))
            erpool = ctx.enter_context(tc.tile_pool(name="er", bufs=ERB))
            dpool = ctx.enter_context(tc.tile_pool(
                name="den", bufs=int(os.environ.get("K_DENB", "3"))))
            opool = ctx.enter_context(tc.tile_pool(name="opix", bufs=OPB))
            otpool = ctx.enter_context(tc.tile_pool(name="ot", bufs=OTBB))
            obpool = ctx.enter_context(tc.tile_pool(name="ob", bufs=OBB))
            xt_view = xt_d.rearrange("(c p) n -> p c n", p=128)

            xcache = {}

            def prefetch_x(blk):
                s0 = blk * 512
                xtile = xin.tile([128, 4, 512], f16, name="xtile", tag="xt")
                xq = nc.gpsimd if XTILE_GP else nc.sync
                xq.dma_start(out=xtile, in_=xt_view[:, :, s0:s0 + 512])
                xcache[blk] = xtile

            def emit_blk(blk, half, part=None):
                s0 = blk * 512
                if half == 0 and part in (None, 0):
                    if blk not in xcache:
                        prefetch_x(blk)
                    emit_blk.cur_x = xcache.pop(blk)
                xtile = emit_blk.cur_x
                ms = (0, 1) if half == 0 else (2, 3)
                if part is not None:
                    ms = (ms[part],)
                for m in ms:
                    # q pair0, q pair1, k pair0, k pair1
                    if os.environ.get("K_QKTAG", "po") == "po":
                        ps = psum.tile([128, 512], f32, tag=po_tag(),
                                       name="psqk")
                    else:
                        ps = psum.tile([128, 512], f32, tag="pw", bufs=2,
                                       name="psqk")
                    for kc in range(4):
                        nc.tensor.matmul(
                            ps,
                            wqkvt[:, kc, m * 128:(m + 1) * 128],
                            xtile[:, kc, :],
                            start=(kc == 0), stop=(kc == 3))
                    # spread the PSUM->SBUF copies across ACT/DVE
                    # (GPSIMD cannot read PSUM)
                    is_q = m < 2
                    dst = (qt[m][:, s0:s0 + 512] if is_q
                           else kt[m - 2][:, 128 + s0:128 + s0 + 512])
                    on_act = is_q == (QKCP == "qa_kd")
                    if on_act:
                        nc.scalar.copy(dst, ps)
                    else:
                        nc.vector.tensor_copy(dst, ps)
                subs = (0, 1) if half == 0 else (2, 3)
                if part is not None:
                    subs = (subs[part],)
                for sub in subs:
                    psv = psum.tile([128, 256], f32, tag=po_tag(),
                                    name="psv")
                    for kc in range(4):
                        nc.tensor.matmul(
                            psv,
                            xtile[:, kc, sub * 128:(sub + 1) * 128],
                            wqkvt[:, kc, 512:768],
                            start=(kc == 0), stop=(kc == 3))
                    ci = 1 + blk * 4 + sub
                    vdst = vsb[:, ci].rearrange("p (h e) -> p h e", h=4)[:, :, 0:64]
                    vsrc = psv.rearrange("p (h e) -> p h e", h=4)
                    v_act = (sub % 2 == 0) if VCP == "mix" else (VCP == "act")
                    if v_act:
                        nc.scalar.copy(vdst, vsrc)
                    else:
                        nc.vector.tensor_copy(vdst, vsrc)

            state = {}

            def emit_front(si):
                s = si * 256
                # em chunk 0 holds the merged edges: cols [0:128) from band
                # chunk 0 (feeds only pixel-half 0), cols [128:256) from band
                # chunk 3 (feeds only half 1). Chunks 1/2 are the full middles.
                # slot order (hs, g): concurrent row-group matmuls (hs=0
                # vs hs=1) must land in different PSUM banks.
                em = epool.tile([128, 3, 4, 256], f16, name="em")

                def mask_bias(pw, c):
                    # pw[j, hs, g, p] = mlog[j, p] (0 or -160) via identity
                    # matmul; the dots then accumulate on top, so the exp
                    # output is masked with no vector work at all.
                    for hs in range(2):
                        nc.tensor.matmul(
                            pw[:, hs, :, :],
                            ident,
                            masks[:, c, :].unsqueeze(1).to_broadcast(
                                [128, 2, 256]),
                            start=True, stop=False, skip_group_check=True)

                def dots_edge():
                    pwe = psum.tile([128, 2, 2, 256], f32, tag="pw",
                                    bufs=2, name="pwe")
                    if FOLD_E:
                        mask_bias(pwe, 0)
                    for g in range(2):
                        for hs in range(2):
                            lo_p, hi_p = hs * 64, (hs + 1) * 64
                            nc.tensor.matmul(
                                pwe[:, hs, g, 0:128],
                                kt[g][lo_p:hi_p, s:s + 128],
                                qt[g][lo_p:hi_p, s:s + 128],
                                start=not FOLD_E, stop=not FOLD_E,
                                skip_group_check=FOLD_E)
                            nc.tensor.matmul(
                                pwe[:, hs, g, 128:256],
                                kt[g][lo_p:hi_p, s + 384:s + 512],
                                qt[g][lo_p:hi_p, s + 128:s + 256],
                                start=not FOLD_E, stop=True,
                                skip_group_check=FOLD_E)
                    return pwe

                def exp_edge(pwe):
                    if FOLD_E:
                        nc.scalar.activation(
                            out=em[:, 0],
                            in_=pwe.rearrange("p a b f -> p (a b) f"),
                            func=Exp, scale=SCALE)
                        return
                    ere = erpool.tile([128, 4, 256], f16, name="ere")
                    nc.scalar.activation(
                        out=ere,
                        in_=pwe.rearrange("p a b f -> p (a b) f"),
                        func=Exp, scale=SCALE)
                    if MASKE == "split":
                        mb = masks01[:, 0, :].unsqueeze(1).to_broadcast(
                            [128, 2, 256])
                        nc.gpsimd.tensor_mul(em[:, 0, 0:2], ere[:, 0:2], mb)
                        nc.vector.tensor_mul(em[:, 0, 2:4], ere[:, 2:4], mb)
                    else:
                        mb = masks01[:, 0, :].unsqueeze(1).to_broadcast(
                            [128, 4, 256])
                        eng = nc.gpsimd if MASKE == "pool" else nc.vector
                        eng.tensor_mul(em[:, 0], ere, mb)

                def mid_chunk(c):
                    pw = psum.tile([128, 2, 2, 256], f32, tag="pw",
                                   bufs=2, name="pw")
                    if FOLD_M:
                        mask_bias(pw, c)
                    for g in range(2):
                        for hs in range(2):
                            lo_p, hi_p = hs * 64, (hs + 1) * 64
                            nc.tensor.matmul(
                                pw[:, hs, g, :],
                                kt[g][lo_p:hi_p, s + 128 * c:s + 128 * c + 128],
                                qt[g][lo_p:hi_p, s:s + 256],
                                start=not FOLD_M, stop=True,
                                skip_group_check=FOLD_M)
                    if FOLD_M:
                        nc.scalar.activation(
                            out=em[:, c],
                            in_=pw.rearrange("p a b f -> p (a b) f"),
                            func=Exp, scale=SCALE)
                    else:
                        er = erpool.tile([128, 4, 256], f16, name="er")
                        nc.scalar.activation(
                            out=er,
                            in_=pw.rearrange("p a b f -> p (a b) f"),
                            func=Exp, scale=SCALE)
                        mb = masks01[:, c, :].unsqueeze(1).to_broadcast(
                            [128, 4, 256])
                        nc.vector.tensor_mul(em[:, c], er, mb)

                if EDGE_FIRST:
                    pwe = dots_edge()
                    exp_edge(pwe)
                    mid_chunk(1)
                    mid_chunk(2)
                else:
                    pwe = dots_edge()
                    mid_chunk(1)
                    mid_chunk(2)
                    exp_edge(pwe)

                if stage < 3:
                    return
                # rotating tags: si+1's po tiles land in other banks, so its
                # AV needn't wait on si's po consumers
                po = [psum.tile([128, 4, 65], f32, tag=po_tag(),
                                name=f"po{ph}") for ph in range(2)]
                for ph in range(2):
                    # edge em chunk 0: cols [0:128) pair with v chunk 2si,
                    # cols [128:256) with v chunk 2si+3
                    cs = ((0, 0), (1, 1), (2, 2)) if ph == 0 \
                        else ((0, 3), (1, 1), (2, 2))
                    for gh in range(4):
                        slot = 2 * (gh % 2) + gh // 2
                        for ci_, (c, vc) in enumerate(cs):
                            nc.tensor.matmul(
                                po[ph][:, gh, 0:65],
                                em[:, c, slot, ph * 128:(ph + 1) * 128],
                                vsb[:, 2 * si + vc, 65 * gh:65 * gh + 65],
                                start=(ci_ == 0), stop=(ci_ == 2))
                if RESC_MERGE:
                    den = dpool.tile([128, 2, 4, 1], f32, tag="den",
                                     name="den")
                    for ph in range(2):
                        nc.vector.tensor_add(
                            den[:, ph], po[ph][:, :, 64:65],
                            npad.unsqueeze(2).to_broadcast([128, 4, 1]))
                    rec = dpool.tile([128, 2, 4, 1], f32, tag="rec",
                                     name="rec")
                    nc.vector.reciprocal(rec, den)
                    recb = dpool.tile([128, 2, 4, 64], f16, tag="recb",
                                      name="recb")
                    nc.vector.tensor_copy(
                        recb, rec.to_broadcast([128, 2, 4, 64]))
                    recs = recb
                else:
                    recs = []
                    for ph in range(2):
                        den = dpool.tile([128, 4, 1], f32, tag=f"den{ph}",
                                         name="den")
                        nc.vector.tensor_add(
                            den, po[ph][:, :, 64:65],
                            npad.unsqueeze(2).to_broadcast([128, 4, 1]))
                        rec = dpool.tile([128, 4, 1], f32, tag=f"rec{ph}",
                                         name="rec")
                        nc.vector.reciprocal(rec, den)
                        recs.append(rec)
                if RESCALE_FRONT:
                    # rescale in the front half: opix is ready well before
                    # the (skewed) back half transposes it
                    state[si] = _rescale(po, recs)
                else:
                    state[si] = (po, recs)

            def _rescale(po, recs):
                opixs = []
                for ph in range(2):
                    opix = opool.tile([128, 256], f16, name="opix")
                    if RESC_MERGE:
                        nc.vector.tensor_mul(
                            opix.rearrange("p (h e) -> p h e", h=4),
                            po[ph][:, :, 0:64],
                            recs[:, ph])
                        opixs.append(opix)
                        continue
                    for gh in range(4):
                        if RESC_ENG == "dve" or (RESC_ENG == "mix" and ph == 0):
                            nc.vector.tensor_scalar_mul(
                                opix[:, gh * 64:(gh + 1) * 64],
                                po[ph][:, gh, 0:64],
                                recs[ph][:, gh, :])
                        else:
                            nc.scalar.mul(
                                opix[:, gh * 64:(gh + 1) * 64],
                                po[ph][:, gh, 0:64],
                                recs[ph][:, gh, :])
                    opixs.append(opix)
                return opixs

            def emit_back(si):
                if stage < 3:
                    return
                s = si * 256
                if RESCALE_FRONT:
                    opixs = state.pop(si)
                else:
                    po, recs = state.pop(si)
                    opixs = _rescale(po, recs)
                if stage < 4:
                    return
                # after the last front, the pw slots are free: tail backs
                # can rotate their transposes through them instead of
                # serializing on the single pt bank
                pt_tag = "pt" if si < int(os.environ.get("K_PTTAIL", "16")) \
                    else "pw"
                if pt_tag == "pw":
                    pt = psum.tile([128, 4, 128], f16, tag="pw", bufs=2,
                                   name="pt")
                else:
                    pt = psum.tile([128, 4, 128], f16, tag="pt", name="pt")
                for ph in range(2):
                    for i in range(2):
                        nc.tensor.transpose(
                            pt[:, 2 * ph + i],
                            opixs[ph][:, i * 128:(i + 1) * 128],
                            ident)
                otb = otpool.tile([128, 4, 128], f16, name="otb")
                if OTB == "dve":
                    nc.vector.tensor_copy(otb, pt)
                elif OTB == "act":
                    nc.scalar.copy(otb, pt)
                else:
                    nc.vector.tensor_copy(otb[:, 0:2], pt[:, 0:2])
                    nc.scalar.copy(otb[:, 2:4], pt[:, 2:4])
                for ph in range(2):
                    # pj reuses the pw slots (rotation: the dots of si+1
                    # were already allocated, so this lands 2 allocs back)
                    pj = psum.tile([128, DIM], f32, tag="pw", bufs=2,
                                   name="pj")
                    for i in range(2):
                        nc.tensor.matmul(
                            pj, otb[:, 2 * ph + i], woutt[:, i],
                            start=(i == 0), stop=(i == 1))
                    # b_out is added host-side during the partial-sum gather.
                    # During the B+C region DVE is saturated -> ob on ACT;
                    # in the C-only tail ACT is the pacer -> split halves.
                    ob = obpool.tile([128, DIM], f16, name="ob")
                    obm = OB if si < int(os.environ.get("K_OBTAIL", "16")) \
                        else os.environ.get("K_OBTAILM", "mix")
                    if obm == "mix":
                        nc.scalar.copy(ob[:, 0:256], pj[:, 0:256])
                        nc.vector.tensor_copy(ob[:, 256:DIM], pj[:, 256:DIM])
                    elif obm == "dve":
                        nc.vector.tensor_copy(ob, pj)
                    else:
                        nc.scalar.copy(ob, pj)
                    px = s + ph * 128
                    oq = nc.sync
                    if (si >= int(os.environ.get("K_ODMA_SI", "16"))
                            and (2 * si + ph) % 2 == 1):
                        oq = nc.scalar
                    oq.dma_start(out=out_d[px:px + 128, :], in_=ob)

            if stage >= 2:
                # Software pipeline: B half-blocks feed si batches (si j needs
                # block <= (j+3)//2); back(si) is emitted after front(si+1) so
                # PE never waits on the rescale chain in program order.
                # early x blocks ride the DMA queue right behind the weight
                # slices they're matmul'd against
                for pb in range(int(os.environ.get("K_XPRE", "1"))):
                    prefetch_x(pb)
                emit_wq_k()
                late_pos = int(os.environ.get("K_LATE", "0"))
                if late_pos == 0:
                    emit_late_consts()
                npre = int(os.environ.get("K_NPRE", "4"))
                pre = [(b, h) for b in range(4) for h in range(2)][:npre]
                for i, (b, h) in enumerate(pre):
                    emit_blk(b, h)
                    if i + 1 == late_pos:
                        emit_late_consts()
                halves = [(b, h) for b in range(4) for h in range(2)][npre:] \
                    + [(b, h) for b in range(4, 8) for h in range(2)]
                straddle = os.environ.get("K_STRAD", "0") == "1"
                for si in range(16):
                    if halves and not HALF_AFTER:
                        emit_blk(*halves.pop(0))
                    if halves and HALF_AFTER and straddle:
                        emit_blk(*halves[0], part=0)
                    emit_front(si)
                    if si >= SKEW:
                        emit_back(si - SKEW)
                    if halves and HALF_AFTER:
                        if straddle:
                            emit_blk(*halves.pop(0), part=1)
                        else:
                            emit_blk(*halves.pop(0))
                for si in range(16 - SKEW, 16):
                    emit_back(si)
            else:
                emit_wq_k()
                emit_late_consts()
                for blk in range(8):
                    emit_blk(blk, 0)
                    emit_blk(blk, 1)
                # dump q instead of attention output
                dbg = consts.tile([128, DIM], f32)
                nc.vector.tensor_copy(dbg, qt[0][:, 0:DIM])
                for t in range(32):
                    nc.sync.dma_start(out=out_d[t * 128:(t + 1) * 128, :], in_=dbg)

    nc.finalize()
    return nc


def _prepare_core_inputs(x, w_qkv, w_out, b_out):
    import os
    mlog, m01, npadt = _make_masks()
    fe = os.environ.get("K_FOLDE", "dve") == "pe"
    fm = os.environ.get("K_FOLDM", "dve") == "pe"
    masks = np.stack([mlog[0] if fe else m01[0],
                      mlog[1] if fm else m01[1],
                      mlog[2] if fm else m01[2]])
    ident = np.eye(128, dtype=np.float16)
    per_core = []
    for ci in range(N_CORES):
        b, hg = ci // 2, ci % 2
        q_rows = w_qkv[256 * hg:256 * hg + 256]
        k_rows = w_qkv[INNER + 256 * hg:INNER + 256 * hg + 256]
        v_rows = w_qkv[2 * INNER + 256 * hg:2 * INNER + 256 * hg + 256]
        w_slice = np.concatenate([q_rows, k_rows, v_rows], axis=0)  # [768, 512]
        per_core.append({
            "xt": np.ascontiguousarray(x[b].T).astype(np.float16),
            "wqkvt": np.ascontiguousarray(w_slice.T).astype(np.float16),
            "woutt": np.ascontiguousarray(
                w_out[:, 256 * hg:256 * hg + 256].T).astype(np.float16),
            "masks": masks,
            "npad": npadt.reshape(128, 1).astype(np.float32),
            "ident": ident,
        })
    return per_core


def kernel(x, w_qkv, w_out, b_out, h, w):
    assert int(h) == HMAP and int(w) == WMAP
    x = np.asarray(x, dtype=np.float32)
    w_qkv = np.asarray(w_qkv, dtype=np.float32)
    w_out = np.asarray(w_out, dtype=np.float32)
    b_out = np.asarray(b_out, dtype=np.float32)

    if "nc" not in _cache:
        _cache["nc"] = _build_nc()
    nc = _cache["nc"]

    from concourse.bass_utils import run_bass_kernel_spmd
    in_maps = _prepare_core_inputs(x, w_qkv, w_out, b_out)
    res = run_bass_kernel_spmd(nc, in_maps, core_ids=list(range(N_CORES)))
    out = np.zeros((B, N, DIM), dtype=np.float32)
    for b in range(B):
        out[b] = (res.results[2 * b]["out"].astype(np.float32)
                  + res.results[2 * b + 1]["out"].astype(np.float32))
    out += b_out  # bias applied host-side once per batch
    return out

